# revision 4
# baseline (speedup 1.0000x reference)
"""LocalRNN (sliding-window GRU) Trainium2 Bass kernel — v4.

Problem: x:[8,2048,768] f32, GRU weights w_ih/w_hh:[768,2304], biases:[2304].
For every position t, run a ksize=8-step GRU over the window x[t-7..t]
(zero left-pad) and emit the final hidden state -> [8,2048,768].
Batch-sharded: one batch item per NeuronCore, 8 cores.

v4 design (vs v3, guided by CoreSim traces + an on-HW matmul probe that
measured 210ns/MM for N=512 fp16 — i.e. the PE runs at the warm roofline
with LDWEIGHTS fully hidden, so the job is keeping PE dense):
- No PSUM gx preloads: r/z matmuls run start=True; gx is added post-matmul
  by one wide DVE tensor_tensor (PSUM+SBUF->SBUF) off the PE critical path.
- Chunk-pair structure: 2 adjacent 512-chunks per (j,d) pass; EW ops run
  once at [128,2,1024]/[128,1024] (halved op overheads).
- All elementwise on DVE + ACT only. An on-HW probe measured GpSimd (Pool)
  traffic slowing the concurrent PE matmul stream by ~32ns/MM (shared SBUF
  ports), while DVE/ACT background load costs ~0 -- so Pool stays idle.
- H update via staged hnew + deferred copy (keeps old-H readers safe while
  halving the staging SBUF vs (g1,t) pairs).
- Phase-1: x cast to fp16 before PE transposes (1 cyc/row), w_ih DMA+cast
  before w_hh (GX starts ~20us earlier), w_hh DMA/cast overlaps GX c2/c3,
  j0 (EW-only) overlaps GX matmuls of later chunks.
"""

import sys
import time

import numpy as np

sys.path.insert(0, "/opt/trn_rl_repo")

import concourse.bass as bass  # noqa: E402
import concourse.tile as tile  # noqa: E402
from concourse import bacc, mybir  # noqa: E402
from concourse.masks import make_identity  # noqa: E402

F32 = mybir.dt.float32
FP16 = mybir.dt.float16
AF = mybir.ActivationFunctionType
OP = mybir.AluOpType

D = 768
G3 = 3 * D
KD = D // 128           # 6 k-tiles
M = G3 // 128           # 18 m-tiles (0-5 r, 6-11 z, 12-17 n)


def build(T=2048, KSIZE=8, CHUNK=512, repeat=1):
    NCH = T // CHUNK            # 4
    NCP = NCH // 2              # 2 chunk-pairs
    CP = 2 * CHUNK              # 1024
    TP = T + KSIZE - 1
    SUB = CHUNK // 128

    nc = bacc.Bacc("TRN2", target_bir_lowering=False, debug=False)
    x = nc.dram_tensor("x", [T, D], F32, kind="ExternalInput").ap()
    w_ih = nc.dram_tensor("w_ih", [D, G3], F32, kind="ExternalInput").ap()
    w_hh = nc.dram_tensor("w_hh", [D, G3], F32, kind="ExternalInput").ap()
    b_ih = nc.dram_tensor("b_ih", [G3], F32, kind="ExternalInput").ap()
    b_hh = nc.dram_tensor("b_hh", [G3], F32, kind="ExternalInput").ap()
    out = nc.dram_tensor("out", [T, D], F32, kind="ExternalOutput").ap()

    with tile.TileContext(nc) as tc:
        with (
            tc.tile_pool(name="perm", bufs=1) as perm,
            tc.tile_pool(name="stage", bufs=KD) as stg,
        ):
            ident_h = perm.tile([128, 128], FP16, name="ident_h")
            make_identity(nc, ident_h[:])

            bih_sb = perm.tile([128, M], F32, name="bih")
            nc.sync.dma_start(bih_sb[:], b_ih.rearrange("(m p) -> p m", p=128))
            bhh_sb = perm.tile([128, M], F32, name="bhh")
            nc.sync.dma_start(bhh_sb[:], b_hh.rearrange("(m p) -> p m", p=128))
            bsum = perm.tile([128, M], F32, name="bsum")
            nc.vector.tensor_tensor(bsum[:], bih_sb[:], bhh_sb[:], op=OP.add)

            whh16 = perm.tile([128, KD, G3], FP16, name="whh16")

            # gx slabs: r/z interleaved [r0,z0,r1,z1,...] so the (r_d, z_d)
            # pair is one [128, 2, n] AP; n gate separate. fp16, biases
            # folded (r/z: b_ih+b_hh; n: b_ih only).
            gxrz = perm.tile([128, 12, TP], FP16, name="gxrz")
            gxn = perm.tile([128, KD, TP], FP16, name="gxn")

            H = perm.tile([128, KD, T], FP16, name="H")

            for rep in range(repeat):
                _emit_once(
                    nc, tc, rep, T, KSIZE, CHUNK, NCH, NCP, CP, TP, SUB,
                    x, w_ih, w_hh, out,
                    ident_h, bih_sb, bhh_sb, bsum,
                    whh16, gxrz, gxn, H, stg,
                )

    nc.compile()
    return nc


def _emit_once(nc, tc, rep, T, KSIZE, CHUNK, NCH, NCP, CP, TP, SUB,
               x, w_ih, w_hh, out,
               ident_h, bih_sb, bhh_sb, bsum,
               whh16, gxrz, gxn, H, stg):
    PAD = KSIZE - 1

    def gslab(m):
        """slab + index for gate-tile m (0..17)."""
        if m < 6:
            return gxrz, 2 * m            # r_d
        if m < 12:
            return gxrz, 2 * (m - 6) + 1  # z_d
        return gxn, m - 12

    def s_tile():
        return stg.tile([128, CP], FP16, name="hnew", tag="hnew")

    def emit_j0_d(cp, d):
        """one d-unit of step 0 (h0=0, no matmul): EW only.
        Uses only outer-pool staging slots so it can interleave anywhere."""
        lo = cp * CP            # j=0 shift is 0
        cs = slice(cp * CP, (cp + 1) * CP)
        r = s_tile()
        nc.scalar.activation(r[:], gxrz[:, 2 * d, lo:lo + CP], AF.Sigmoid)
        z = s_tile()
        nc.scalar.activation(z[:], gxrz[:, 2 * d + 1, lo:lo + CP],
                             AF.Sigmoid)
        g1 = s_tile()
        nc.vector.scalar_tensor_tensor(
            g1[:], r[:], bhh_sb[:, d + 12:d + 13],
            gxn[:, d, lo:lo + CP], op0=OP.mult, op1=OP.add)
        nc.scalar.activation(g1[:], g1[:], AF.Tanh)
        u = s_tile()
        nc.vector.tensor_tensor(u[:], z[:], g1[:], op=OP.mult)
        nc.vector.tensor_tensor(H[:, d, cs], g1[:], u[:], op=OP.subtract)

    # ------- phase W + 1: weight DMA/cast; GX = w_ih.T@X.T + biases;
    # ------- j=0 interleaved per chunk-pair (EW only, hides under PE)
    with (
        tc.tile_pool(name=f"wih{rep}", bufs=1) as wip,
        tc.tile_pool(name=f"xload{rep}", bufs=2) as xp,
        tc.tile_pool(name=f"xc{rep}", bufs=2) as xcp,
        tc.tile_pool(name=f"xt{rep}", bufs=2) as xtp,
        tc.tile_pool(name=f"pad{rep}", bufs=1) as padp,
        tc.tile_pool(name=f"pst{rep}", bufs=3, space="PSUM") as ps_t,
        tc.tile_pool(name=f"psg{rep}", bufs=5, space="PSUM") as ps_g,
    ):
        wih16 = wip.tile([128, KD, G3], FP16, name="wih16")
        WPC = 384  # weight cast piece (f32 staging kept small)

        def load_w(dst, src):
            # stage f32 pieces through the (otherwise idle) 6-slot stage
            # pool: deep enough that DMA streams ahead of the DVE casts
            # instead of ping-ponging on a 2-slot round trip.
            for k in range(KD):
                for p in range(G3 // WPC):
                    wt = stg.tile([128, WPC], F32, name="wt", tag="hnew")
                    nc.sync.dma_start(
                        wt[:], src[k * 128:(k + 1) * 128,
                                   p * WPC:(p + 1) * WPC])
                    nc.vector.tensor_copy(
                        dst[:, k, p * WPC:(p + 1) * WPC], wt[:])

        def load_sub(c, i, xts):
            """DMA + cast + transpose sub-block i of chunk c into xts."""
            xn = xp.tile([128, D], F32, name="xn")
            t0 = c * CHUNK + i * 128
            nc.sync.dma_start(xn[:], x[t0:t0 + 128, :])
            xh = xcp.tile([128, D], FP16, name="xh")
            nc.vector.tensor_copy(xh[:], xn[:])
            for k2 in range(KD // 2):
                pt = ps_t.tile([128, 2, 128], FP16, name="pt")
                for q in range(2):
                    k = 2 * k2 + q
                    nc.tensor.transpose(
                        pt[:, q, :], xh[:, k * 128:(k + 1) * 128],
                        ident_h[:])
                nc.vector.tensor_copy(
                    xts[:, 2 * k2:2 * k2 + 2, i * 128:(i + 1) * 128],
                    pt[:])

        def load_chunk(c, xts):
            for i in range(SUB):
                load_sub(c, i, xts)

        def gx_chunk(c, xts, interleave=()):
            """GX matmuls for chunk c; `interleave` callbacks are emitted
            every 5 m-groups so PE/ACT/DVE FIFOs alternate finely instead
            of clumping (a clump of transposes stalls PE on evac rate)."""
            ivs = list(interleave)
            for m in range(M):
                if m % 5 == 0 and ivs:
                    ivs.pop(0)()
                pg = ps_g.tile([128, CHUNK], F32, name="pg")
                for k in range(KD):
                    nc.tensor.matmul(
                        pg[:], wih16[:, k, m * 128:(m + 1) * 128],
                        xts[:, k, :],
                        start=(k == 0), stop=(k == KD - 1),
                    )
                slab, si = gslab(m)
                col = bsum if m < 12 else bih_sb
                nc.scalar.activation(
                    slab[:, si, PAD + c * CHUNK:PAD + (c + 1) * CHUNK],
                    pg[:], AF.Identity, bias=col[:, m:m + 1])
            for iv in ivs:
                iv()

        xts_all = [xtp.tile([128, KD, CHUNK], FP16, name="xts", tag="xts")
                   for c in range(NCH)]
        # chunks 0/1 up front (PE transposes warm up while w_ih streams in)
        load_chunk(0, xts_all[0])
        load_chunk(1, xts_all[1])
        load_w(wih16, w_ih)

        # left-pad region: gx = bias only (zero input contribution)
        zt = padp.tile([128, PAD], F32, name="padzero")
        nc.vector.memset(zt[:], 0.0)
        for m in range(M):
            slab, si = gslab(m)
            col = bsum if m < 12 else bih_sb
            nc.scalar.activation(slab[:, si, 0:PAD], zt[:], AF.Identity,
                                 bias=col[:, m:m + 1])

        # one-chunk lookahead: load c+1 rides inside gx_c (xts slot of
        # chunk c+1 is free once gx_{c-1} finished -- safe with 2 bufs).
        gx_chunk(0, xts_all[0],
                 interleave=[lambda i=i: load_sub(2, i, xts_all[2])
                             for i in range(SUB)])
        gx_chunk(1, xts_all[1],
                 interleave=[lambda i=i: load_sub(3, i, xts_all[3])
                             for i in range(SUB)])
        # w_hh DMA + cast: overlaps GX matmuls of chunks 2/3
        load_w(whh16, w_hh)
        # j0 for cpair 0 rides inside gx2/gx3; j0 for cpair 1 is
        # interleaved into phase-2's first step (see below).
        gx_chunk(2, xts_all[2],
                 interleave=[lambda d=d: emit_j0_d(0, d) for d in (0, 1, 2)])
        gx_chunk(3, xts_all[3],
                 interleave=[lambda d=d: emit_j0_d(0, d) for d in (3, 4, 5)])

    # ---------------- phase 2: steps 1..KSIZE-1 -----------------------
    with (
        tc.tile_pool(name=f"ew{rep}", bufs=2) as ewp,
        tc.tile_pool(name=f"ost{rep}", bufs=2 * SUB) as ostp,
        tc.tile_pool(name=f"rzps{rep}", bufs=1, space="PSUM") as rz_pp,
        tc.tile_pool(name=f"nps{rep}", bufs=1, space="PSUM") as n_pp,
        tc.tile_pool(name=f"pso{rep}", bufs=2, space="PSUM") as ps_o,
    ):
        rz_ps = rz_pp.tile([128, 2, CP], F32, name="rz_ps")
        n_ps = n_pp.tile([128, CP], F32, name="n_ps")

        def emit_out(og, hn, d):
            """transpose one d-slab of a finished hnew into the og stage."""
            for i in range(2 * SUB):
                po = ps_o.tile([128, 128], FP16, name="po")
                nc.tensor.transpose(
                    po[:], hn[:, i * 128:(i + 1) * 128], ident_h[:])
                nc.scalar.activation(
                    og[i][:, d * 128:(d + 1) * 128], po[:], AF.Copy)

        for j in range(1, KSIZE):
            for cp in range(NCP):
                cs = slice(cp * CP, (cp + 1) * CP)
                lo = j + cp * CP      # window into padded gx time axis
                last = (j == KSIZE - 1)
                if j == 1 and cp == 0:
                    # j0 for cpair 1: 3 units fit the ACT slack before
                    # j1cp0's first sigmoid; the rest ride inline below.
                    for d0 in (0, 1, 2):
                        emit_j0_d(1, d0)
                og = ([ostp.tile([128, D], F32, name="og", tag="og")
                       for i in range(2 * SUB)] if last else None)
                hnews = []
                pending = None
                for d in range(KD):
                    # --- matmuls: 3 gates x 6 k x 2 chunks of 512 ---
                    # r/z groups start with an identity matmul that injects
                    # gx (w/ folded biases) into the accumulator: kills the
                    # two big DVE PSUM-adds per iteration; sigmoid then
                    # reads PSUM directly.
                    for g in range(3):
                        m = d + 6 * g
                        o = (rz_ps[:, g, :] if g < 2 else n_ps[:])
                        if g < 2:
                            for c in range(2):
                                lc = lo + c * CHUNK
                                nc.tensor.matmul(
                                    o[:, c * CHUNK:(c + 1) * CHUNK],
                                    ident_h[:],
                                    gxrz[:, 2 * d + g, lc:lc + CHUNK],
                                    start=True, stop=False,
                                    skip_group_check=True,
                                )
                        for kk in range(KD):
                            for c in range(2):
                                nc.tensor.matmul(
                                    o[:, c * CHUNK:(c + 1) * CHUNK],
                                    whh16[:, kk, m * 128:(m + 1) * 128],
                                    H[:, kk,
                                      (2 * cp + c) * CHUNK:
                                      (2 * cp + c + 1) * CHUNK],
                                    start=(g == 2 and kk == 0),
                                    stop=(kk == KD - 1),
                                    skip_group_check=True,
                                )
                    # j=7: PE emits output transposes for hnew[d-2] (lag
                    # keeps the PE FIFO from stalling on the EW chain)
                    if last and d >= 2:
                        emit_out(og, hnews[d - 2][1], d - 2)

                    # --- EW epilogue (2 chunks wide) ---
                    # sigmoids read PSUM directly (gx already accumulated);
                    # split r/z so each bank pair frees ASAP (next d's
                    # start=True identity matmuls wait on these reads)
                    rz = ewp.tile([128, 2, CP], FP16, name="rz")
                    nc.scalar.activation(rz[:, 0, :], rz_ps[:, 0, :],
                                         AF.Sigmoid)
                    nc.scalar.activation(rz[:, 1, :], rz_ps[:, 1, :],
                                         AF.Sigmoid)
                    if pending is not None:
                        pending()
                    g1 = ewp.tile([128, CP], FP16, name="g1")
                    nc.vector.scalar_tensor_tensor(
                        g1[:], n_ps[:], bhh_sb[:, d + 12:d + 13],
                        rz[:, 0, :], op0=OP.add, op1=OP.mult)
                    nc.vector.tensor_tensor(
                        g1[:], g1[:], gxn[:, d, lo:lo + CP], op=OP.add)
                    nc.scalar.activation(g1[:], g1[:], AF.Tanh)
                    t = ewp.tile([128, CP], FP16, name="t")
                    nc.vector.tensor_tensor(t[:], H[:, d, cs], g1[:],
                                            op=OP.subtract)

                    def mk_tail(d=d, rz=rz, g1=g1, t=t):
                        def fin():
                            nc.vector.tensor_tensor(t[:], rz[:, 1, :], t[:],
                                                    op=OP.mult)
                            hn = s_tile()
                            nc.vector.tensor_tensor(hn[:], g1[:], t[:],
                                                    op=OP.add)
                            hnews.append((d, hn))
                        return fin
                    pending = mk_tail()

                    # j0 for cpair 1 rides the slack of step (j=1, cp=0).
                    # Only 3 units here: hnew staging (6 slots) holds d+1
                    # live hnews + 3 transient j0 tiles at the peak.
                    if j == 1 and cp == 0 and d < 3:
                        emit_j0_d(1, d + 3)
                pending()

                if last:
                    # drain remaining output transposes; hnew IS the final
                    # h for this cpair -- H is dead, skip the copies.
                    emit_out(og, hnews[4][1], 4)
                    emit_out(og, hnews[5][1], 5)
                    for i in range(2 * SUB):
                        t0 = cp * CP + i * 128
                        nc.sync.dma_start(out[t0:t0 + 128, :], og[i][:])
                else:
                    # deferred H update: all of this cpair's matmuls (and
                    # the subtracts) read old H; Tile orders the copies
                    # after them
                    for d, hn in hnews:
                        nc.vector.tensor_copy(H[:, d, cs], hn[:])


# --------------------------------------------------------------------------
# PJRT runner (resident buffers, jit built once)
# --------------------------------------------------------------------------
class BassRunner:
    def __init__(self, nc, n_cores: int):
        import jax
        from jax.sharding import Mesh, PartitionSpec
        from jax.experimental.shard_map import shard_map
        from concourse.bass2jax import (
            _bass_exec_p, install_neuronx_cc_hook, partition_id_tensor,
        )

        install_neuronx_cc_hook()
        self.jax = jax
        self.nc = nc
        self.n_cores = n_cores

        partition_name = (
            nc.partition_id_tensor.name if nc.partition_id_tensor else None
        )
        in_names, out_names, out_avals, zero_outs = [], [], [], []
        for alloc in nc.m.functions[0].allocations:
            if not isinstance(alloc, mybir.MemoryLocationSet):
                continue
            name = alloc.memorylocations[0].name
            if alloc.kind == "ExternalInput":
                if name != partition_name:
                    in_names.append(name)
            elif alloc.kind == "ExternalOutput":
                shape = tuple(alloc.tensor_shape)
                dtype = mybir.dt.np(alloc.dtype)
                out_names.append(name)
                out_avals.append(jax.core.ShapedArray(shape, dtype))
                zero_outs.append(np.zeros(shape, dtype))
        self.in_names = in_names
        self.out_names = out_names
        self.zero_outs = zero_outs
        n_params = len(in_names)
        all_in_names = list(in_names) + list(out_names)
        if partition_name is not None:
            all_in_names.append(partition_name)

        def _body(*args):
            operands = list(args)
            if partition_name is not None:
                operands.append(partition_id_tensor())
            outs = _bass_exec_p.bind(
                *operands,
                out_avals=tuple(out_avals),
                in_names=tuple(all_in_names),
                out_names=tuple(out_names),
                lowering_input_output_aliases=(),
                sim_require_finite=True,
                sim_require_nnan=True,
                nc=nc,
            )
            return tuple(outs)

        devices = jax.devices()[:n_cores]
        assert len(devices) == n_cores, (
            f"need {n_cores} neuron devices, have {len(jax.devices())}"
        )
        if n_cores == 1:
            self.fn = jax.jit(_body, keep_unused=True)
        else:
            mesh = Mesh(np.asarray(devices), ("core",))
            in_specs = (PartitionSpec("core"),) * (n_params + len(out_names))
            out_specs = (PartitionSpec("core"),) * len(out_names)
            self.fn = jax.jit(
                shard_map(_body, mesh=mesh, in_specs=in_specs,
                          out_specs=out_specs, check_rep=False),
                keep_unused=True,
            )
        self._dev_args = None

    def stage(self, in_maps):
        assert len(in_maps) == self.n_cores
        if self.n_cores == 1:
            concat = [np.asarray(in_maps[0][n]) for n in self.in_names]
            concat += list(self.zero_outs)
        else:
            concat = [
                np.concatenate([np.asarray(m[n]) for m in in_maps], axis=0)
                for n in self.in_names
            ]
            concat += [
                np.concatenate([z] * self.n_cores, axis=0)
                for z in self.zero_outs
            ]
        self._dev_args = self.jax.device_put(concat)
        self.jax.block_until_ready(self._dev_args)

    def run(self):
        outs = self.fn(*self._dev_args)
        self.jax.block_until_ready(outs)
        return outs

    def run_results(self):
        outs = self.run()
        per_core = [{} for _ in range(self.n_cores)]
        for name, arr in zip(self.out_names, outs):
            arr = np.asarray(arr)
            if self.n_cores == 1:
                per_core[0][name] = arr
            else:
                for c, s in enumerate(np.split(arr, self.n_cores, axis=0)):
                    per_core[c][name] = s
        return per_core

    def time_runs(self, iters=10, warmup=2):
        for _ in range(warmup):
            self.run()
        ts = []
        for _ in range(iters):
            t0 = time.perf_counter()
            self.run()
            ts.append(time.perf_counter() - t0)
        return ts


_CACHE = {}


def _get_runner(T, KSIZE, n_cores, repeat=1):
    key = (T, KSIZE, n_cores, repeat)
    if key not in _CACHE:
        nc = build(T=T, KSIZE=KSIZE, repeat=repeat)
        _CACHE[key] = BassRunner(nc, n_cores)
    return _CACHE[key]


def kernel(x, w_ih, w_hh, b_ih, b_hh, ksize):
    x = np.ascontiguousarray(np.asarray(x, dtype=np.float32))
    B, T, _D = x.shape
    ksize = int(ksize)
    runner = _get_runner(T, ksize, B)
    w_ih = np.ascontiguousarray(np.asarray(w_ih, dtype=np.float32))
    w_hh = np.ascontiguousarray(np.asarray(w_hh, dtype=np.float32))
    b_ih = np.ascontiguousarray(np.asarray(b_ih, dtype=np.float32))
    b_hh = np.ascontiguousarray(np.asarray(b_hh, dtype=np.float32))
    in_maps = [
        {"x": x[b], "w_ih": w_ih, "w_hh": w_hh, "b_ih": b_ih, "b_hh": b_hh}
        for b in range(B)
    ]
    runner.stage(in_maps)
    res = runner.run_results()
    return np.stack([res[b]["out"] for b in range(B)], axis=0)


# revision 6
# speedup vs baseline: 1.0326x; 1.0326x over previous
"""LocalRNN (sliding-window GRU) Trainium2 Bass kernel — v4.

Problem: x:[8,2048,768] f32, GRU weights w_ih/w_hh:[768,2304], biases:[2304].
For every position t, run a ksize=8-step GRU over the window x[t-7..t]
(zero left-pad) and emit the final hidden state -> [8,2048,768].
Batch-sharded: one batch item per NeuronCore, 8 cores.

v4 design (vs v3, guided by CoreSim traces + an on-HW matmul probe that
measured 210ns/MM for N=512 fp16 — i.e. the PE runs at the warm roofline
with LDWEIGHTS fully hidden, so the job is keeping PE dense):
- No PSUM gx preloads: r/z matmuls run start=True; gx is added post-matmul
  by one wide DVE tensor_tensor (PSUM+SBUF->SBUF) off the PE critical path.
- Chunk-pair structure: 2 adjacent 512-chunks per (j,d) pass; EW ops run
  once at [128,2,1024]/[128,1024] (halved op overheads).
- All elementwise on DVE + ACT only. An on-HW probe measured GpSimd (Pool)
  traffic slowing the concurrent PE matmul stream by ~32ns/MM (shared SBUF
  ports), while DVE/ACT background load costs ~0 -- so Pool stays idle.
- H update via staged hnew + deferred copy (keeps old-H readers safe while
  halving the staging SBUF vs (g1,t) pairs).
- Phase-1: x cast to fp16 before PE transposes (1 cyc/row), w_ih DMA+cast
  before w_hh (GX starts ~20us earlier), w_hh DMA/cast overlaps GX c2/c3,
  j0 (EW-only) overlaps GX matmuls of later chunks.
"""

import sys
import time

import numpy as np

sys.path.insert(0, "/opt/trn_rl_repo")

import concourse.bass as bass  # noqa: E402
import concourse.tile as tile  # noqa: E402
from concourse import bacc, mybir  # noqa: E402
from concourse.masks import make_identity  # noqa: E402

F32 = mybir.dt.float32
FP16 = mybir.dt.float16
AF = mybir.ActivationFunctionType
OP = mybir.AluOpType

D = 768
G3 = 3 * D
KD = D // 128           # 6 k-tiles
M = G3 // 128           # 18 m-tiles (0-5 r, 6-11 z, 12-17 n)


def build(T=2048, KSIZE=8, CHUNK=512, repeat=1):
    NCH = T // CHUNK            # 4
    NCP = NCH // 2              # 2 chunk-pairs
    CP = 2 * CHUNK              # 1024
    TP = T + KSIZE - 1
    SUB = CHUNK // 128

    nc = bacc.Bacc("TRN2", target_bir_lowering=False, debug=False)
    x = nc.dram_tensor("x", [T, D], F32, kind="ExternalInput").ap()
    w_ih = nc.dram_tensor("w_ih", [D, G3], F32, kind="ExternalInput").ap()
    w_hh = nc.dram_tensor("w_hh", [D, G3], F32, kind="ExternalInput").ap()
    b_ih = nc.dram_tensor("b_ih", [G3], F32, kind="ExternalInput").ap()
    b_hh = nc.dram_tensor("b_hh", [G3], F32, kind="ExternalInput").ap()
    out = nc.dram_tensor("out", [T, D], F32, kind="ExternalOutput").ap()

    with tile.TileContext(nc) as tc:
        with (
            tc.tile_pool(name="perm", bufs=1) as perm,
            tc.tile_pool(name="stage", bufs=KD) as stg,
        ):
            ident_h = perm.tile([128, 128], FP16, name="ident_h")
            make_identity(nc, ident_h[:])

            bih_sb = perm.tile([128, M], F32, name="bih")
            nc.sync.dma_start(bih_sb[:], b_ih.rearrange("(m p) -> p m", p=128))
            bhh_sb = perm.tile([128, M], F32, name="bhh")
            nc.sync.dma_start(bhh_sb[:], b_hh.rearrange("(m p) -> p m", p=128))
            bsum = perm.tile([128, M], F32, name="bsum")
            nc.vector.tensor_tensor(bsum[:], bih_sb[:], bhh_sb[:], op=OP.add)

            whh16 = perm.tile([128, KD, G3], FP16, name="whh16")

            # gx slabs: r/z interleaved [r0,z0,r1,z1,...] so the (r_d, z_d)
            # pair is one [128, 2, n] AP; n gate separate. fp16, biases
            # folded (r/z: b_ih+b_hh; n: b_ih only).
            gxrz = perm.tile([128, 12, TP], FP16, name="gxrz")
            gxn = perm.tile([128, KD, TP], FP16, name="gxn")

            H = perm.tile([128, KD, T], FP16, name="H")

            for rep in range(repeat):
                _emit_once(
                    nc, tc, rep, T, KSIZE, CHUNK, NCH, NCP, CP, TP, SUB,
                    x, w_ih, w_hh, out,
                    ident_h, bih_sb, bhh_sb, bsum,
                    whh16, gxrz, gxn, H, stg,
                )

    nc.compile()
    return nc


def _emit_once(nc, tc, rep, T, KSIZE, CHUNK, NCH, NCP, CP, TP, SUB,
               x, w_ih, w_hh, out,
               ident_h, bih_sb, bhh_sb, bsum,
               whh16, gxrz, gxn, H, stg):
    PAD = KSIZE - 1

    def gslab(m):
        """slab + index for gate-tile m (0..17)."""
        if m < 6:
            return gxrz, 2 * m            # r_d
        if m < 12:
            return gxrz, 2 * (m - 6) + 1  # z_d
        return gxn, m - 12

    def s_tile():
        return stg.tile([128, CP], FP16, name="hnew", tag="hnew")

    def emit_j0_d(cp, d):
        """one d-unit of step 0 (h0=0, no matmul): EW only.
        Uses only outer-pool staging slots so it can interleave anywhere."""
        lo = cp * CP            # j=0 shift is 0
        cs = slice(cp * CP, (cp + 1) * CP)
        r = s_tile()
        nc.scalar.activation(r[:], gxrz[:, 2 * d, lo:lo + CP], AF.Sigmoid)
        z = s_tile()
        nc.scalar.activation(z[:], gxrz[:, 2 * d + 1, lo:lo + CP],
                             AF.Sigmoid)
        g1 = s_tile()
        nc.vector.scalar_tensor_tensor(
            g1[:], r[:], bhh_sb[:, d + 12:d + 13],
            gxn[:, d, lo:lo + CP], op0=OP.mult, op1=OP.add)
        nc.scalar.activation(g1[:], g1[:], AF.Tanh)
        u = s_tile()
        nc.vector.tensor_tensor(u[:], z[:], g1[:], op=OP.mult)
        nc.vector.tensor_tensor(H[:, d, cs], g1[:], u[:], op=OP.subtract)

    # ------- phase W + 1: weight DMA/cast; GX = w_ih.T@X.T + biases;
    # ------- j=0 interleaved per chunk-pair (EW only, hides under PE)
    with (
        tc.tile_pool(name=f"wih{rep}", bufs=1) as wip,
        tc.tile_pool(name=f"xload{rep}", bufs=2) as xp,
        tc.tile_pool(name=f"xc{rep}", bufs=2) as xcp,
        tc.tile_pool(name=f"xt{rep}", bufs=2) as xtp,
        tc.tile_pool(name=f"pad{rep}", bufs=1) as padp,
        tc.tile_pool(name=f"pst{rep}", bufs=3, space="PSUM") as ps_t,
        tc.tile_pool(name=f"psg{rep}", bufs=5, space="PSUM") as ps_g,
    ):
        wih16 = wip.tile([128, KD, G3], FP16, name="wih16")
        WPC = 384  # weight cast piece (f32 staging kept small)

        def load_w(dst, src):
            # stage f32 pieces through the (otherwise idle) 6-slot stage
            # pool: deep enough that DMA streams ahead of the DVE casts
            # instead of ping-ponging on a 2-slot round trip. Column-band
            # (p) major: after one band, all 6 k-tiles of m-tiles
            # [3p, 3p+3) are resident, so GX starts ~6x earlier.
            for p in range(G3 // WPC):
                for k in range(KD):
                    wt = stg.tile([128, WPC], F32, name="wt", tag="hnew")
                    nc.sync.dma_start(
                        wt[:], src[k * 128:(k + 1) * 128,
                                   p * WPC:(p + 1) * WPC])
                    nc.vector.tensor_copy(
                        dst[:, k, p * WPC:(p + 1) * WPC], wt[:])

        def load_sub(c, i, xts):
            """DMA + cast + transpose sub-block i of chunk c into xts."""
            xn = xp.tile([128, D], F32, name="xn")
            t0 = c * CHUNK + i * 128
            nc.sync.dma_start(xn[:], x[t0:t0 + 128, :])
            xh = xcp.tile([128, D], FP16, name="xh")
            nc.vector.tensor_copy(xh[:], xn[:])
            for k2 in range(KD // 2):
                pt = ps_t.tile([128, 2, 128], FP16, name="pt")
                for q in range(2):
                    k = 2 * k2 + q
                    nc.tensor.transpose(
                        pt[:, q, :], xh[:, k * 128:(k + 1) * 128],
                        ident_h[:])
                nc.vector.tensor_copy(
                    xts[:, 2 * k2:2 * k2 + 2, i * 128:(i + 1) * 128],
                    pt[:])

        def load_chunk(c, xts):
            for i in range(SUB):
                load_sub(c, i, xts)

        def gx_chunk(c, xts, interleave=()):
            """GX matmuls for chunk c; `interleave` callbacks are emitted
            every 5 m-groups so PE/ACT/DVE FIFOs alternate finely instead
            of clumping (a clump of transposes stalls PE on evac rate)."""
            ivs = list(interleave)
            for m in range(M):
                if m % 5 == 0 and ivs:
                    ivs.pop(0)()
                pg = ps_g.tile([128, CHUNK], F32, name="pg")
                for k in range(KD):
                    nc.tensor.matmul(
                        pg[:], wih16[:, k, m * 128:(m + 1) * 128],
                        xts[:, k, :],
                        start=(k == 0), stop=(k == KD - 1),
                    )
                slab, si = gslab(m)
                col = bsum if m < 12 else bih_sb
                nc.scalar.activation(
                    slab[:, si, PAD + c * CHUNK:PAD + (c + 1) * CHUNK],
                    pg[:], AF.Identity, bias=col[:, m:m + 1])
            for iv in ivs:
                iv()

        xts_all = [xtp.tile([128, KD, CHUNK], FP16, name="xts", tag="xts")
                   for c in range(NCH)]
        # chunks 0/1 up front (PE transposes warm up while w_ih streams in)
        load_chunk(0, xts_all[0])
        load_chunk(1, xts_all[1])
        load_w(wih16, w_ih)

        # left-pad region: gx = bias only (zero input contribution)
        zt = padp.tile([128, PAD], F32, name="padzero")
        nc.vector.memset(zt[:], 0.0)
        for m in range(M):
            slab, si = gslab(m)
            col = bsum if m < 12 else bih_sb
            nc.scalar.activation(slab[:, si, 0:PAD], zt[:], AF.Identity,
                                 bias=col[:, m:m + 1])

        # one-chunk lookahead: load c+1 rides inside gx_c (xts slot of
        # chunk c+1 is free once gx_{c-1} finished -- safe with 2 bufs).
        gx_chunk(0, xts_all[0],
                 interleave=[lambda i=i: load_sub(2, i, xts_all[2])
                             for i in range(SUB)])
        gx_chunk(1, xts_all[1],
                 interleave=[lambda i=i: load_sub(3, i, xts_all[3])
                             for i in range(SUB)])
        # w_hh DMA + cast: overlaps GX matmuls of chunks 2/3
        load_w(whh16, w_hh)
        # j0 for cpair 0 rides inside gx2/gx3; j0 for cpair 1 is
        # interleaved into phase-2's first step (see below).
        gx_chunk(2, xts_all[2],
                 interleave=[lambda d=d: emit_j0_d(0, d) for d in (0, 1, 2)])
        gx_chunk(3, xts_all[3],
                 interleave=[lambda d=d: emit_j0_d(0, d) for d in (3, 4, 5)])

    # ---------------- phase 2: steps 1..KSIZE-1 -----------------------
    with (
        tc.tile_pool(name=f"ew{rep}", bufs=2) as ewp,
        tc.tile_pool(name=f"ost{rep}", bufs=2 * SUB) as ostp,
        tc.tile_pool(name=f"rzps{rep}", bufs=1, space="PSUM") as rz_pp,
        tc.tile_pool(name=f"nps{rep}", bufs=1, space="PSUM") as n_pp,
        tc.tile_pool(name=f"pso{rep}", bufs=2, space="PSUM") as ps_o,
    ):
        rz_ps = rz_pp.tile([128, 2, CP], F32, name="rz_ps")
        n_ps = n_pp.tile([128, CP], F32, name="n_ps")

        def emit_out(og, hn, d):
            """transpose one d-slab of a finished hnew into the og stage."""
            for i in range(2 * SUB):
                po = ps_o.tile([128, 128], FP16, name="po")
                nc.tensor.transpose(
                    po[:], hn[:, i * 128:(i + 1) * 128], ident_h[:])
                nc.scalar.activation(
                    og[i][:, d * 128:(d + 1) * 128], po[:], AF.Copy)

        for j in range(1, KSIZE):
            for cp in range(NCP):
                cs = slice(cp * CP, (cp + 1) * CP)
                lo = j + cp * CP      # window into padded gx time axis
                last = (j == KSIZE - 1)
                if j == 1 and cp == 0:
                    # j0 for cpair 1: 3 units fit the ACT slack before
                    # j1cp0's first sigmoid; the rest ride inline below.
                    for d0 in (0, 1, 2):
                        emit_j0_d(1, d0)
                og = ([ostp.tile([128, D], F32, name="og", tag="og")
                       for i in range(2 * SUB)] if last else None)
                hnews = []
                pending = None
                for d in range(KD):
                    # previous d's EW tail is emitted BEFORE this d's
                    # matmuls: its stt must read n_ps ahead of the n-group
                    # rewrite below, and its ops are all ready when the
                    # engines reach them, so the strict-FIFO DVE queue
                    # never blocks on a not-yet-ready instruction ahead of
                    # the bank-freeing adds.
                    if pending is not None:
                        pending()
                    # --- matmuls: 3 gates x 6 k x 2 chunks of 512 ---
                    for g in range(3):
                        m = d + 6 * g
                        o = (rz_ps[:, g, :] if g < 2 else n_ps[:])
                        for kk in range(KD):
                            for c in range(2):
                                nc.tensor.matmul(
                                    o[:, c * CHUNK:(c + 1) * CHUNK],
                                    whh16[:, kk, m * 128:(m + 1) * 128],
                                    H[:, kk,
                                      (2 * cp + c) * CHUNK:
                                      (2 * cp + c + 1) * CHUNK],
                                    start=(kk == 0), stop=(kk == KD - 1),
                                    skip_group_check=True,
                                )
                    # j=7: PE emits output transposes for hnew[d-2] (lag
                    # keeps the PE FIFO from stalling on the EW chain)
                    if last and d >= 2:
                        emit_out(og, hnews[d - 2][1], d - 2)

                    # --- EW head (2 chunks wide) ---
                    # r/z adds split so each PSUM bank pair frees ASAP
                    # (next d's start=True matmuls wait on these reads);
                    # sigmoids split likewise so r is ready early for stt.
                    rz = ewp.tile([128, 2, CP], FP16, name="rz")
                    nc.vector.tensor_tensor(
                        rz[:, 0, :], rz_ps[:, 0, :],
                        gxrz[:, 2 * d, lo:lo + CP], op=OP.add)
                    nc.vector.tensor_tensor(
                        rz[:, 1, :], rz_ps[:, 1, :],
                        gxrz[:, 2 * d + 1, lo:lo + CP], op=OP.add)
                    nc.scalar.activation(rz[:, 0, :], rz[:, 0, :],
                                         AF.Sigmoid)
                    nc.scalar.activation(rz[:, 1, :], rz[:, 1, :],
                                         AF.Sigmoid)

                    def mk_tail(d=d, rz=rz):
                        def fin():
                            g1 = ewp.tile([128, CP], FP16, name="g1")
                            nc.vector.scalar_tensor_tensor(
                                g1[:], n_ps[:], bhh_sb[:, d + 12:d + 13],
                                rz[:, 0, :], op0=OP.add, op1=OP.mult)
                            nc.vector.tensor_tensor(
                                g1[:], g1[:], gxn[:, d, lo:lo + CP],
                                op=OP.add)
                            nc.scalar.activation(g1[:], g1[:], AF.Tanh)
                            t = ewp.tile([128, CP], FP16, name="t")
                            nc.vector.tensor_tensor(t[:], H[:, d, cs],
                                                    g1[:], op=OP.subtract)
                            nc.vector.tensor_tensor(t[:], rz[:, 1, :], t[:],
                                                    op=OP.mult)
                            hn = s_tile()
                            nc.vector.tensor_tensor(hn[:], g1[:], t[:],
                                                    op=OP.add)
                            hnews.append((d, hn))
                        return fin
                    pending = mk_tail()

                    # j0 for cpair 1 rides the slack of step (j=1, cp=0).
                    # Only 3 units here: hnew staging (6 slots) holds d+1
                    # live hnews + 3 transient j0 tiles at the peak.
                    if j == 1 and cp == 0 and d < 3:
                        emit_j0_d(1, d + 3)
                pending()

                if last:
                    # drain remaining output transposes; hnew IS the final
                    # h for this cpair -- H is dead, skip the copies.
                    emit_out(og, hnews[4][1], 4)
                    emit_out(og, hnews[5][1], 5)
                    for i in range(2 * SUB):
                        t0 = cp * CP + i * 128
                        nc.sync.dma_start(out[t0:t0 + 128, :], og[i][:])
                else:
                    # deferred H update: all of this cpair's matmuls (and
                    # the subtracts) read old H; Tile orders the copies
                    # after them
                    # H copies on ACT (idle-ish; keeps DVE for the adds)
                    for d, hn in hnews:
                        nc.scalar.activation(H[:, d, cs], hn[:], AF.Copy)


# --------------------------------------------------------------------------
# PJRT runner (resident buffers, jit built once)
# --------------------------------------------------------------------------
class BassRunner:
    def __init__(self, nc, n_cores: int):
        import jax
        from jax.sharding import Mesh, PartitionSpec
        from jax.experimental.shard_map import shard_map
        from concourse.bass2jax import (
            _bass_exec_p, install_neuronx_cc_hook, partition_id_tensor,
        )

        install_neuronx_cc_hook()
        self.jax = jax
        self.nc = nc
        self.n_cores = n_cores

        partition_name = (
            nc.partition_id_tensor.name if nc.partition_id_tensor else None
        )
        in_names, out_names, out_avals, zero_outs = [], [], [], []
        for alloc in nc.m.functions[0].allocations:
            if not isinstance(alloc, mybir.MemoryLocationSet):
                continue
            name = alloc.memorylocations[0].name
            if alloc.kind == "ExternalInput":
                if name != partition_name:
                    in_names.append(name)
            elif alloc.kind == "ExternalOutput":
                shape = tuple(alloc.tensor_shape)
                dtype = mybir.dt.np(alloc.dtype)
                out_names.append(name)
                out_avals.append(jax.core.ShapedArray(shape, dtype))
                zero_outs.append(np.zeros(shape, dtype))
        self.in_names = in_names
        self.out_names = out_names
        self.zero_outs = zero_outs
        n_params = len(in_names)
        all_in_names = list(in_names) + list(out_names)
        if partition_name is not None:
            all_in_names.append(partition_name)

        def _body(*args):
            operands = list(args)
            if partition_name is not None:
                operands.append(partition_id_tensor())
            outs = _bass_exec_p.bind(
                *operands,
                out_avals=tuple(out_avals),
                in_names=tuple(all_in_names),
                out_names=tuple(out_names),
                lowering_input_output_aliases=(),
                sim_require_finite=True,
                sim_require_nnan=True,
                nc=nc,
            )
            return tuple(outs)

        devices = jax.devices()[:n_cores]
        assert len(devices) == n_cores, (
            f"need {n_cores} neuron devices, have {len(jax.devices())}"
        )
        if n_cores == 1:
            self.fn = jax.jit(_body, keep_unused=True)
        else:
            mesh = Mesh(np.asarray(devices), ("core",))
            in_specs = (PartitionSpec("core"),) * (n_params + len(out_names))
            out_specs = (PartitionSpec("core"),) * len(out_names)
            self.fn = jax.jit(
                shard_map(_body, mesh=mesh, in_specs=in_specs,
                          out_specs=out_specs, check_rep=False),
                keep_unused=True,
            )
        self._dev_args = None

    def stage(self, in_maps):
        assert len(in_maps) == self.n_cores
        if self.n_cores == 1:
            concat = [np.asarray(in_maps[0][n]) for n in self.in_names]
            concat += list(self.zero_outs)
        else:
            concat = [
                np.concatenate([np.asarray(m[n]) for m in in_maps], axis=0)
                for n in self.in_names
            ]
            concat += [
                np.concatenate([z] * self.n_cores, axis=0)
                for z in self.zero_outs
            ]
        self._dev_args = self.jax.device_put(concat)
        self.jax.block_until_ready(self._dev_args)

    def run(self):
        outs = self.fn(*self._dev_args)
        self.jax.block_until_ready(outs)
        return outs

    def run_results(self):
        outs = self.run()
        per_core = [{} for _ in range(self.n_cores)]
        for name, arr in zip(self.out_names, outs):
            arr = np.asarray(arr)
            if self.n_cores == 1:
                per_core[0][name] = arr
            else:
                for c, s in enumerate(np.split(arr, self.n_cores, axis=0)):
                    per_core[c][name] = s
        return per_core

    def time_runs(self, iters=10, warmup=2):
        for _ in range(warmup):
            self.run()
        ts = []
        for _ in range(iters):
            t0 = time.perf_counter()
            self.run()
            ts.append(time.perf_counter() - t0)
        return ts


_CACHE = {}


def _get_runner(T, KSIZE, n_cores, repeat=1):
    key = (T, KSIZE, n_cores, repeat)
    if key not in _CACHE:
        nc = build(T=T, KSIZE=KSIZE, repeat=repeat)
        _CACHE[key] = BassRunner(nc, n_cores)
    return _CACHE[key]


def kernel(x, w_ih, w_hh, b_ih, b_hh, ksize):
    x = np.ascontiguousarray(np.asarray(x, dtype=np.float32))
    B, T, _D = x.shape
    ksize = int(ksize)
    runner = _get_runner(T, ksize, B)
    w_ih = np.ascontiguousarray(np.asarray(w_ih, dtype=np.float32))
    w_hh = np.ascontiguousarray(np.asarray(w_hh, dtype=np.float32))
    b_ih = np.ascontiguousarray(np.asarray(b_ih, dtype=np.float32))
    b_hh = np.ascontiguousarray(np.asarray(b_hh, dtype=np.float32))
    in_maps = [
        {"x": x[b], "w_ih": w_ih, "w_hh": w_hh, "b_ih": b_ih, "b_hh": b_hh}
        for b in range(B)
    ]
    runner.stage(in_maps)
    res = runner.run_results()
    return np.stack([res[b]["out"] for b in range(B)], axis=0)


# revision 7
# speedup vs baseline: 1.1487x; 1.1124x over previous
"""LocalRNN (sliding-window GRU) Trainium2 Bass kernel — v4.

Problem: x:[8,2048,768] f32, GRU weights w_ih/w_hh:[768,2304], biases:[2304].
For every position t, run a ksize=8-step GRU over the window x[t-7..t]
(zero left-pad) and emit the final hidden state -> [8,2048,768].
Batch-sharded: one batch item per NeuronCore, 8 cores.

v4 design (vs v3, guided by CoreSim traces + an on-HW matmul probe that
measured 210ns/MM for N=512 fp16 — i.e. the PE runs at the warm roofline
with LDWEIGHTS fully hidden, so the job is keeping PE dense):
- No PSUM gx preloads: r/z matmuls run start=True; gx is added post-matmul
  by one wide DVE tensor_tensor (PSUM+SBUF->SBUF) off the PE critical path.
- Chunk-pair structure: 2 adjacent 512-chunks per (j,d) pass; EW ops run
  once at [128,2,1024]/[128,1024] (halved op overheads).
- All elementwise on DVE + ACT only. An on-HW probe measured GpSimd (Pool)
  traffic slowing the concurrent PE matmul stream by ~32ns/MM (shared SBUF
  ports), while DVE/ACT background load costs ~0 -- so Pool stays idle.
- H update via staged hnew + deferred copy (keeps old-H readers safe while
  halving the staging SBUF vs (g1,t) pairs).
- Phase-1: x cast to fp16 before PE transposes (1 cyc/row), w_ih DMA+cast
  before w_hh (GX starts ~20us earlier), w_hh DMA/cast overlaps GX c2/c3,
  j0 (EW-only) overlaps GX matmuls of later chunks.
"""

import sys
import time

import numpy as np

sys.path.insert(0, "/opt/trn_rl_repo")

import concourse.bass as bass  # noqa: E402
import concourse.tile as tile  # noqa: E402
from concourse import bacc, mybir  # noqa: E402
from concourse.masks import make_identity  # noqa: E402

F32 = mybir.dt.float32
FP16 = mybir.dt.float16
AF = mybir.ActivationFunctionType
OP = mybir.AluOpType

D = 768
G3 = 3 * D
KD = D // 128           # 6 k-tiles
M = G3 // 128           # 18 m-tiles (0-5 r, 6-11 z, 12-17 n)


def build(T=2048, KSIZE=8, CHUNK=512, repeat=1):
    NCH = T // CHUNK            # 4
    NCP = NCH // 2              # 2 chunk-pairs
    CP = 2 * CHUNK              # 1024
    TP = T + KSIZE - 1
    SUB = CHUNK // 128

    nc = bacc.Bacc("TRN2", target_bir_lowering=False, debug=False)
    x = nc.dram_tensor("x", [T, D], F32, kind="ExternalInput").ap()
    w_ih = nc.dram_tensor("w_ih", [D, G3], F32, kind="ExternalInput").ap()
    w_hh = nc.dram_tensor("w_hh", [D, G3], F32, kind="ExternalInput").ap()
    b_ih = nc.dram_tensor("b_ih", [G3], F32, kind="ExternalInput").ap()
    b_hh = nc.dram_tensor("b_hh", [G3], F32, kind="ExternalInput").ap()
    out = nc.dram_tensor("out", [T, D], F32, kind="ExternalOutput").ap()

    with tile.TileContext(nc) as tc:
        with (
            tc.tile_pool(name="perm", bufs=1) as perm,
            tc.tile_pool(name="stage", bufs=KD) as stg,
        ):
            ident_h = perm.tile([128, 128], FP16, name="ident_h")
            make_identity(nc, ident_h[:])

            bih_sb = perm.tile([128, M], F32, name="bih")
            nc.sync.dma_start(bih_sb[:], b_ih.rearrange("(m p) -> p m", p=128))
            bhh_sb = perm.tile([128, M], F32, name="bhh")
            nc.sync.dma_start(bhh_sb[:], b_hh.rearrange("(m p) -> p m", p=128))
            bsum = perm.tile([128, M], F32, name="bsum")
            nc.vector.tensor_tensor(bsum[:], bih_sb[:], bhh_sb[:], op=OP.add)

            whh16 = perm.tile([128, KD, G3], FP16, name="whh16")

            # gx slabs: r/z interleaved [r0,z0,r1,z1,...] so the (r_d, z_d)
            # pair is one [128, 2, n] AP; n gate separate. fp16, biases
            # folded (r/z: b_ih+b_hh; n: b_ih only).
            gxrz = perm.tile([128, 12, TP], FP16, name="gxrz")
            gxn = perm.tile([128, KD, TP], FP16, name="gxn")

            H = perm.tile([128, KD, T], FP16, name="H")

            for rep in range(repeat):
                _emit_once(
                    nc, tc, rep, T, KSIZE, CHUNK, NCH, NCP, CP, TP, SUB,
                    x, w_ih, w_hh, out,
                    ident_h, bih_sb, bhh_sb, bsum,
                    whh16, gxrz, gxn, H, stg,
                )

    nc.compile()
    return nc


def _emit_once(nc, tc, rep, T, KSIZE, CHUNK, NCH, NCP, CP, TP, SUB,
               x, w_ih, w_hh, out,
               ident_h, bih_sb, bhh_sb, bsum,
               whh16, gxrz, gxn, H, stg):
    PAD = KSIZE - 1

    def gslab(m):
        """slab + index for gate-tile m (0..17)."""
        if m < 6:
            return gxrz, 2 * m            # r_d
        if m < 12:
            return gxrz, 2 * (m - 6) + 1  # z_d
        return gxn, m - 12

    def s_tile():
        return stg.tile([128, CP], FP16, name="hnew", tag="hnew")

    def emit_j0_d(cp, d):
        """one d-unit of step 0 (h0=0, no matmul): EW only.
        Uses only outer-pool staging slots so it can interleave anywhere."""
        lo = cp * CP            # j=0 shift is 0
        cs = slice(cp * CP, (cp + 1) * CP)
        r = s_tile()
        nc.scalar.activation(r[:], gxrz[:, 2 * d, lo:lo + CP], AF.Sigmoid)
        z = s_tile()
        nc.scalar.activation(z[:], gxrz[:, 2 * d + 1, lo:lo + CP],
                             AF.Sigmoid)
        g1 = s_tile()
        nc.vector.scalar_tensor_tensor(
            g1[:], r[:], bhh_sb[:, d + 12:d + 13],
            gxn[:, d, lo:lo + CP], op0=OP.mult, op1=OP.add)
        nc.scalar.activation(g1[:], g1[:], AF.Tanh)
        u = s_tile()
        nc.vector.tensor_tensor(u[:], z[:], g1[:], op=OP.mult)
        nc.vector.tensor_tensor(H[:, d, cs], g1[:], u[:], op=OP.subtract)

    # ------- phase W + 1: weight DMA/cast; GX = w_ih.T@X.T + biases;
    # ------- j=0 interleaved per chunk-pair (EW only, hides under PE)
    with (
        tc.tile_pool(name=f"wih{rep}", bufs=1) as wip,
        tc.tile_pool(name=f"xload{rep}", bufs=2) as xp,
        tc.tile_pool(name=f"xc{rep}", bufs=2) as xcp,
        tc.tile_pool(name=f"xt{rep}", bufs=2) as xtp,
        tc.tile_pool(name=f"pad{rep}", bufs=1) as padp,
        tc.tile_pool(name=f"pst{rep}", bufs=3, space="PSUM") as ps_t,
        tc.tile_pool(name=f"psg{rep}", bufs=5, space="PSUM") as ps_g,
    ):
        wih16 = wip.tile([128, KD, G3], FP16, name="wih16")
        WPC = 384  # weight cast piece (f32 staging kept small)

        def load_w(dst, src):
            # stage f32 pieces through the (otherwise idle) 6-slot stage
            # pool: deep enough that DMA streams ahead of the DVE casts
            # instead of ping-ponging on a 2-slot round trip. Column-band
            # (p) major: after one band, all 6 k-tiles of m-tiles
            # [3p, 3p+3) are resident, so GX can start ~6x earlier.
            for p in range(G3 // WPC):
                for k in range(KD):
                    wt = stg.tile([128, WPC], F32, name="wt", tag="hnew")
                    nc.sync.dma_start(
                        wt[:], src[k * 128:(k + 1) * 128,
                                   p * WPC:(p + 1) * WPC])
                    nc.vector.tensor_copy(
                        dst[:, k, p * WPC:(p + 1) * WPC], wt[:])

        def load_sub(c, i, xts):
            """DMA + cast + transpose sub-block i of chunk c into xts."""
            xn = xp.tile([128, D], F32, name="xn")
            t0 = c * CHUNK + i * 128
            nc.sync.dma_start(xn[:], x[t0:t0 + 128, :])
            xh = xcp.tile([128, D], FP16, name="xh")
            nc.vector.tensor_copy(xh[:], xn[:])
            for k2 in range(KD // 2):
                pt = ps_t.tile([128, 2, 128], FP16, name="pt")
                for q in range(2):
                    k = 2 * k2 + q
                    nc.tensor.transpose(
                        pt[:, q, :], xh[:, k * 128:(k + 1) * 128],
                        ident_h[:])
                nc.vector.tensor_copy(
                    xts[:, 2 * k2:2 * k2 + 2, i * 128:(i + 1) * 128],
                    pt[:])

        def load_chunk(c, xts):
            for i in range(SUB):
                load_sub(c, i, xts)

        def gx_chunk(c, xts, interleave=(), step=5):
            """GX matmuls for chunk c; `interleave` callbacks are emitted
            every `step` m-groups so PE/ACT/DVE FIFOs alternate finely
            instead of clumping (a clump of transposes stalls PE on evac
            rate)."""
            ivs = list(interleave)
            for m in range(M):
                if m % step == 0 and ivs:
                    ivs.pop(0)()
                pg = ps_g.tile([128, CHUNK], F32, name="pg")
                for k in range(KD):
                    nc.tensor.matmul(
                        pg[:], wih16[:, k, m * 128:(m + 1) * 128],
                        xts[:, k, :],
                        start=(k == 0), stop=(k == KD - 1),
                    )
                slab, si = gslab(m)
                col = bsum if m < 12 else bih_sb
                nc.scalar.activation(
                    slab[:, si, PAD + c * CHUNK:PAD + (c + 1) * CHUNK],
                    pg[:], AF.Identity, bias=col[:, m:m + 1])
            for iv in ivs:
                iv()

        xts_all = [xtp.tile([128, KD, CHUNK], FP16, name="xts", tag="xts")
                   for c in range(NCH)]
        # chunks 0/1 up front (PE transposes warm up while w_ih streams in)
        load_chunk(0, xts_all[0])
        load_chunk(1, xts_all[1])
        load_w(wih16, w_ih)

        # left-pad region: gx = bias only (zero input contribution)
        zt = padp.tile([128, PAD], F32, name="padzero")
        nc.vector.memset(zt[:], 0.0)
        for m in range(M):
            slab, si = gslab(m)
            col = bsum if m < 12 else bih_sb
            nc.scalar.activation(slab[:, si, 0:PAD], zt[:], AF.Identity,
                                 bias=col[:, m:m + 1])

        # one-chunk lookahead: load c+1 rides inside gx_c (xts slot of
        # chunk c+1 is free once gx_{c-1} finished -- safe with 2 bufs).
        gx_chunk(0, xts_all[0],
                 interleave=[lambda i=i: load_sub(2, i, xts_all[2])
                             for i in range(SUB)])
        gx_chunk(1, xts_all[1],
                 interleave=[lambda i=i: load_sub(3, i, xts_all[3])
                             for i in range(SUB)])
        # w_hh DMA + cast: overlaps GX matmuls of chunks 2/3
        load_w(whh16, w_hh)
        # j0 for cpair 0 rides inside gx2/gx3; j0 for cpair 1 is
        # interleaved into phase-2's first step (see below).
        gx_chunk(2, xts_all[2],
                 interleave=[lambda d=d: emit_j0_d(0, d) for d in (0, 1, 2)])
        # step=3: the last unit's H write lands ~2.5us earlier, so the
        # first phase-2 matmul group (which reads all H d-slabs) isn't
        # left waiting on H[:,5,cp0].
        gx_chunk(3, xts_all[3],
                 interleave=[lambda d=d: emit_j0_d(0, d) for d in (3, 4, 5)],
                 step=3)

    # ---------------- phase 2: steps 1..KSIZE-1 -----------------------
    with (
        tc.tile_pool(name=f"ew{rep}", bufs=2) as ewp,
        tc.tile_pool(name=f"ost{rep}", bufs=2 * SUB) as ostp,
        tc.tile_pool(name=f"rzps{rep}", bufs=1, space="PSUM") as rz_pp,
        tc.tile_pool(name=f"nps{rep}", bufs=1, space="PSUM") as n_pp,
        tc.tile_pool(name=f"pso{rep}", bufs=2, space="PSUM") as ps_o,
    ):
        rz_ps = rz_pp.tile([128, 2, CP], F32, name="rz_ps")
        n_ps = n_pp.tile([128, CP], F32, name="n_ps")

        def emit_out(og, hn, d):
            """transpose one d-slab of a finished hnew into the og stage."""
            for i in range(2 * SUB):
                po = ps_o.tile([128, 128], FP16, name="po")
                nc.tensor.transpose(
                    po[:], hn[:, i * 128:(i + 1) * 128], ident_h[:])
                nc.scalar.activation(
                    og[i][:, d * 128:(d + 1) * 128], po[:], AF.Copy)

        for j in range(1, KSIZE):
            for cp in range(NCP):
                cs = slice(cp * CP, (cp + 1) * CP)
                lo = j + cp * CP      # window into padded gx time axis
                last = (j == KSIZE - 1)
                if j == 1 and cp == 0:
                    # j0 for cpair 1: 3 units fit the ACT slack before
                    # j1cp0's first sigmoid; the rest ride inline below.
                    for d0 in (0, 1, 2):
                        emit_j0_d(1, d0)
                og = ([ostp.tile([128, D], F32, name="og", tag="og")
                       for i in range(2 * SUB)] if last else None)
                hnews = []
                pending = None
                for d in range(KD):
                    # --- matmuls: 3 gates x 6 k x 2 chunks of 512 ---
                    for g in range(3):
                        m = d + 6 * g
                        o = (rz_ps[:, g, :] if g < 2 else n_ps[:])
                        for kk in range(KD):
                            for c in range(2):
                                nc.tensor.matmul(
                                    o[:, c * CHUNK:(c + 1) * CHUNK],
                                    whh16[:, kk, m * 128:(m + 1) * 128],
                                    H[:, kk,
                                      (2 * cp + c) * CHUNK:
                                      (2 * cp + c + 1) * CHUNK],
                                    start=(kk == 0), stop=(kk == KD - 1),
                                    skip_group_check=True,
                                )
                    # j=7: PE emits output transposes for hnew[d-2] (lag
                    # keeps the PE FIFO from stalling on the EW chain)
                    if last and d >= 2:
                        emit_out(og, hnews[d - 2][1], d - 2)

                    # --- EW epilogue (2 chunks wide) ---
                    # r/z adds split so each PSUM bank pair frees ASAP
                    # (next d's start=True matmuls wait on these reads)
                    rz = ewp.tile([128, 2, CP], FP16, name="rz")
                    nc.vector.tensor_tensor(
                        rz[:, 0, :], rz_ps[:, 0, :],
                        gxrz[:, 2 * d, lo:lo + CP], op=OP.add)
                    nc.vector.tensor_tensor(
                        rz[:, 1, :], rz_ps[:, 1, :],
                        gxrz[:, 2 * d + 1, lo:lo + CP], op=OP.add)
                    # previous d's DVE tail (mul + hnew) is emitted AFTER
                    # this d's bank-freeing adds: keeps the adds at the DVE
                    # queue head so the next matmul group never waits.
                    if pending is not None:
                        pending()
                    nc.scalar.activation(rz[:], rz[:], AF.Sigmoid)
                    g1 = ewp.tile([128, CP], FP16, name="g1")
                    nc.vector.scalar_tensor_tensor(
                        g1[:], n_ps[:], bhh_sb[:, d + 12:d + 13],
                        rz[:, 0, :], op0=OP.add, op1=OP.mult)
                    nc.vector.tensor_tensor(
                        g1[:], g1[:], gxn[:, d, lo:lo + CP], op=OP.add)
                    nc.scalar.activation(g1[:], g1[:], AF.Tanh)
                    t = ewp.tile([128, CP], FP16, name="t")
                    nc.vector.tensor_tensor(t[:], H[:, d, cs], g1[:],
                                            op=OP.subtract)

                    def mk_tail(d=d, rz=rz, g1=g1, t=t):
                        def fin():
                            nc.vector.tensor_tensor(t[:], rz[:, 1, :], t[:],
                                                    op=OP.mult)
                            hn = s_tile()
                            nc.vector.tensor_tensor(hn[:], g1[:], t[:],
                                                    op=OP.add)
                            hnews.append((d, hn))
                        return fin
                    pending = mk_tail()

                    # j0 for cpair 1 rides the slack of step (j=1, cp=0).
                    # Only 3 units here: hnew staging (6 slots) holds d+1
                    # live hnews + 3 transient j0 tiles at the peak.
                    if j == 1 and cp == 0 and d < 3:
                        emit_j0_d(1, d + 3)
                pending()

                if last:
                    # drain remaining output transposes; hnew IS the final
                    # h for this cpair -- H is dead, skip the copies.
                    emit_out(og, hnews[4][1], 4)
                    emit_out(og, hnews[5][1], 5)
                    for i in range(2 * SUB):
                        t0 = cp * CP + i * 128
                        nc.sync.dma_start(out[t0:t0 + 128, :], og[i][:])
                else:
                    # deferred H update: all of this cpair's matmuls (and
                    # the subtracts) read old H; Tile orders the copies
                    # after them
                    for d, hn in hnews:
                        nc.vector.tensor_copy(H[:, d, cs], hn[:])


# --------------------------------------------------------------------------
# PJRT runner (resident buffers, jit built once)
# --------------------------------------------------------------------------
class BassRunner:
    def __init__(self, nc, n_cores: int):
        import jax
        from jax.sharding import Mesh, PartitionSpec
        from jax.experimental.shard_map import shard_map
        from concourse.bass2jax import (
            _bass_exec_p, install_neuronx_cc_hook, partition_id_tensor,
        )

        install_neuronx_cc_hook()
        self.jax = jax
        self.nc = nc
        self.n_cores = n_cores

        partition_name = (
            nc.partition_id_tensor.name if nc.partition_id_tensor else None
        )
        in_names, out_names, out_avals, zero_outs = [], [], [], []
        for alloc in nc.m.functions[0].allocations:
            if not isinstance(alloc, mybir.MemoryLocationSet):
                continue
            name = alloc.memorylocations[0].name
            if alloc.kind == "ExternalInput":
                if name != partition_name:
                    in_names.append(name)
            elif alloc.kind == "ExternalOutput":
                shape = tuple(alloc.tensor_shape)
                dtype = mybir.dt.np(alloc.dtype)
                out_names.append(name)
                out_avals.append(jax.core.ShapedArray(shape, dtype))
                zero_outs.append(np.zeros(shape, dtype))
        self.in_names = in_names
        self.out_names = out_names
        self.zero_outs = zero_outs
        n_params = len(in_names)
        all_in_names = list(in_names) + list(out_names)
        if partition_name is not None:
            all_in_names.append(partition_name)

        def _body(*args):
            operands = list(args)
            if partition_name is not None:
                operands.append(partition_id_tensor())
            outs = _bass_exec_p.bind(
                *operands,
                out_avals=tuple(out_avals),
                in_names=tuple(all_in_names),
                out_names=tuple(out_names),
                lowering_input_output_aliases=(),
                sim_require_finite=True,
                sim_require_nnan=True,
                nc=nc,
            )
            return tuple(outs)

        devices = jax.devices()[:n_cores]
        assert len(devices) == n_cores, (
            f"need {n_cores} neuron devices, have {len(jax.devices())}"
        )
        if n_cores == 1:
            self.fn = jax.jit(_body, keep_unused=True)
        else:
            mesh = Mesh(np.asarray(devices), ("core",))
            in_specs = (PartitionSpec("core"),) * (n_params + len(out_names))
            out_specs = (PartitionSpec("core"),) * len(out_names)
            self.fn = jax.jit(
                shard_map(_body, mesh=mesh, in_specs=in_specs,
                          out_specs=out_specs, check_rep=False),
                keep_unused=True,
            )
        self._dev_args = None

    def stage(self, in_maps):
        assert len(in_maps) == self.n_cores
        if self.n_cores == 1:
            concat = [np.asarray(in_maps[0][n]) for n in self.in_names]
            concat += list(self.zero_outs)
        else:
            concat = [
                np.concatenate([np.asarray(m[n]) for m in in_maps], axis=0)
                for n in self.in_names
            ]
            concat += [
                np.concatenate([z] * self.n_cores, axis=0)
                for z in self.zero_outs
            ]
        self._dev_args = self.jax.device_put(concat)
        self.jax.block_until_ready(self._dev_args)

    def run(self):
        outs = self.fn(*self._dev_args)
        self.jax.block_until_ready(outs)
        return outs

    def run_results(self):
        outs = self.run()
        per_core = [{} for _ in range(self.n_cores)]
        for name, arr in zip(self.out_names, outs):
            arr = np.asarray(arr)
            if self.n_cores == 1:
                per_core[0][name] = arr
            else:
                for c, s in enumerate(np.split(arr, self.n_cores, axis=0)):
                    per_core[c][name] = s
        return per_core

    def time_runs(self, iters=10, warmup=2):
        for _ in range(warmup):
            self.run()
        ts = []
        for _ in range(iters):
            t0 = time.perf_counter()
            self.run()
            ts.append(time.perf_counter() - t0)
        return ts


_CACHE = {}


def _get_runner(T, KSIZE, n_cores, repeat=1):
    key = (T, KSIZE, n_cores, repeat)
    if key not in _CACHE:
        nc = build(T=T, KSIZE=KSIZE, repeat=repeat)
        _CACHE[key] = BassRunner(nc, n_cores)
    return _CACHE[key]


def kernel(x, w_ih, w_hh, b_ih, b_hh, ksize):
    x = np.ascontiguousarray(np.asarray(x, dtype=np.float32))
    B, T, _D = x.shape
    ksize = int(ksize)
    runner = _get_runner(T, ksize, B)
    w_ih = np.ascontiguousarray(np.asarray(w_ih, dtype=np.float32))
    w_hh = np.ascontiguousarray(np.asarray(w_hh, dtype=np.float32))
    b_ih = np.ascontiguousarray(np.asarray(b_ih, dtype=np.float32))
    b_hh = np.ascontiguousarray(np.asarray(b_hh, dtype=np.float32))
    in_maps = [
        {"x": x[b], "w_ih": w_ih, "w_hh": w_hh, "b_ih": b_ih, "b_hh": b_hh}
        for b in range(B)
    ]
    runner.stage(in_maps)
    res = runner.run_results()
    return np.stack([res[b]["out"] for b in range(B)], axis=0)
